# revision 24
# baseline (speedup 1.0000x reference)
"""Trainium2 Bass kernel: GQA attention block (S=2048, HID=4096, 32 q heads /
8 kv heads, head dim 128, RoPE, causal), tensor-parallel over heads on 8
NeuronCores.

Sharding: core c owns q heads [4c..4c+4) and kv head c. wq/wk/wv are sharded
on their output dim, wo on its input dim; each core computes a partial
y_c = o_c @ wo_c.T and the host sums the 8 partials (the "all-reduce").

v3 (bf16 + double-bank PSUM tiles): all matmul operands are bf16 (PSUM fp32);
PSUM is organized as three rotating groups of [128,1024] tiles (2 banks each,
individual matmuls write 512-col halves) plus a [128,512] scratch tag:

  phase 1  q01/q23/kv accumulate pairwise in the 1024-wide tiles
  phase 2  runs one head at a time over 1024-query chunks: the j-loop is
           software-pipelined (scores j ahead of AV/den j-1), exp covers the
           whole [128,<=1024] block in ONE ACT instruction, the causal
           diagonal is a DVE triangular-mask multiply, and the softmax
           denominators for both query halves pack into partitions 0/32 of a
           single PSUM bank via matmul tile_position
  phase 3  y accumulates per 1024-query pair; a single DVE copy drains each

PSUM->SBUF drains all run on DVE (measured ~3x faster than ACT for fp32
PSUM reads); ACT keeps exp + the RoPE half-swaps; Pool does the RoPE
add/sub tails and bulk DMA issue so the Sync queue only carries output.
"""

import os
import sys

import numpy as np

for _p in (
    "/root/.axon_site",
    "/root/.axon_site/_ro/trn_rl_repo",
    "/root/.axon_site/_ro/pypackages",
    "/opt/trn_rl_repo",
):
    if os.path.isdir(_p) and _p not in sys.path:
        sys.path.append(_p)

import concourse.bacc as bacc  # noqa: E402
import concourse.mybir as mybir  # noqa: E402
from concourse import bass_utils  # noqa: E402
from concourse.tile import TileContext  # noqa: E402

F32 = mybir.dt.float32
BF16 = mybir.dt.bfloat16

N_CORES = 8
SEQ = 2048
HID = 4096
NQ = 32
NKV = 8
HD = 128
THETA = 500000.0

HQ = NQ // N_CORES  # 4 q heads per core
QC = HQ * HD  # 512: per-core q feature slice
KB = SEQ // 128  # 16 key blocks
NKBLK = HID // 128  # 32 contraction blocks for the projections
NBIG = SEQ // 1024  # 2 big (1024-query) chunks
SCALE = 1.0 / float(np.sqrt(HD))


def _build_body(tc, sb, sbw, ps):
    nc = tc.nc

    xT = nc.dram_tensor("xT", (HID, SEQ), BF16, kind="ExternalInput").ap()
    wq_sb_d = nc.dram_tensor("wq_sb", (128, NKBLK * QC), BF16, kind="ExternalInput").ap()
    wk_sb_d = nc.dram_tensor("wk_sb", (128, NKBLK * HD), BF16, kind="ExternalInput").ap()
    wv_sb_d = nc.dram_tensor("wv_sb", (128, NKBLK * HD), BF16, kind="ExternalInput").ap()
    wo_sb_d = nc.dram_tensor("wo_sb", (128, 4 * HID), BF16, kind="ExternalInput").ap()
    ones_d = nc.dram_tensor("ones_in", (128, 128), BF16, kind="ExternalInput").ap()
    tri_d = nc.dram_tensor("tri_in", (128, 128), BF16, kind="ExternalInput").ap()
    cc_d = nc.dram_tensor("cc", (HD, SEQ), F32, kind="ExternalInput").ap()
    ss_d = nc.dram_tensor("ss", (HD, SEQ), F32, kind="ExternalInput").ap()
    yT_d = nc.dram_tensor("yT", (HID, SEQ), BF16, kind="ExternalOutput").ap()
    dscr = nc.dram_tensor("den_scratch", (17, 1024), F32).ap()

    # --- persistent SBUF tiles ---
    ones = sb.tile([128, 128], BF16, name="ones")
    tri = sb.tile([128, 128], BF16, name="tri")
    ccf = sb.tile([128, SEQ], F32, name="ccf")
    ssf = sb.tile([128, SEQ], F32, name="ssf")
    wq_t = sb.tile([128, NKBLK * QC], BF16, name="wq_t")
    wk_t = sb.tile([128, NKBLK * HD], BF16, name="wk_t")
    wv_t = sb.tile([128, NKBLK * HD], BF16, name="wv_t")
    # a small first weight piece so the k-loop can start almost immediately;
    # the rest streams from inside the loop. Tables go on the ACT/Pool queues
    # so the Sync queue stays clear for the x stream.
    nc.sync.dma_start(wq_t[:, 0 : 2 * QC], wq_sb_d[:, 0 : 2 * QC])
    nc.sync.dma_start(wk_t[:, 0 : 2 * HD], wk_sb_d[:, 0 : 2 * HD])
    nc.sync.dma_start(wv_t[:, 0 : 2 * HD], wv_sb_d[:, 0 : 2 * HD])
    nc.scalar.dma_start(wk_t[:, 2 * HD : 16 * HD], wk_sb_d[:, 2 * HD : 16 * HD])
    nc.scalar.dma_start(wv_t[:, 2 * HD : 16 * HD], wv_sb_d[:, 2 * HD : 16 * HD])
    nc.scalar.dma_start(ccf[:], cc_d[:])
    nc.scalar.dma_start(ssf[:], ss_d[:])
    nc.gpsimd.dma_start(ones[:], ones_d[:])
    nc.gpsimd.dma_start(tri[:], tri_d[:])

    # PE warmup: dummy matmuls on a vector-memset tile so the HAM clock gate
    # opens before the first real matmul; kept alive by a tiny DMA.
    warm_in = sb.tile([128, 512], BF16, name="warm_in")
    nc.vector.memset(warm_in[:], 0.5)
    warm_ps = ps.tile([128, 512], F32, tag="den", bufs=2, name="warm_ps")
    for wi in range(16):
        nc.tensor.matmul(warm_ps[:], warm_in[:, 0:128], warm_in[:], start=(wi == 0), stop=(wi == 15))
    warm_sb = sbw.tile([1, 512], F32, tag="den", bufs=4, name="warm_sb")
    nc.vector.tensor_copy(warm_sb[0:1, :], warm_ps[0:1, :])
    nc.sync.dma_start(dscr[0:1, 0:512], warm_sb[0:1, :])

    qT = [sb.tile([128, SEQ], BF16, name=f"qT{h}") for h in range(HQ)]
    kT = sb.tile([128, SEQ], BF16, name="kT")
    vnat = sb.tile([128, KB * 128], BF16, name="vnat")

    # =================== phase 1: QKV projections + RoPE ===================
    xT_r = xT.rearrange("(c p) s -> p c s", p=128)  # [128, 32, SEQ]

    def rope_drain(acc):
        """PSUM->SBUF drain on DVE (frees the PSUM bank for the next chunk)."""
        cpy = sbw.tile([128, 512], F32, tag="ropetmp", bufs=12, name="cpy")
        nc.vector.tensor_copy(cpy[:], acc[:])
        return cpy

    def rope_finish(dst, cpy, s0):
        """dst[:, s0:s0+512] = rope(cpy); partition rows 0:64 hold even rope
        dims, 64:128 odd (host permuted the weight rows). ACT builds the
        half-swap and the odd-dim add; DVE does the products and even sub."""
        sw = sbw.tile([128, 512], F32, tag="ropetmp", bufs=12, name="sw")
        nc.scalar.copy(sw[0:64, :], cpy[64:128, :])
        nc.scalar.copy(sw[64:128, :], cpy[0:64, :])
        m1 = sbw.tile([128, 512], F32, tag="ropetmp", bufs=12, name="m1")
        m2 = sbw.tile([128, 512], F32, tag="ropetmp", bufs=12, name="m2")
        nc.vector.tensor_mul(m1[:], cpy[:], ccf[:, s0 : s0 + 512])
        nc.vector.tensor_mul(m2[:], sw[:], ssf[:, s0 : s0 + 512])
        nc.vector.tensor_sub(dst[0:64, s0 : s0 + 512], m1[0:64, :], m2[0:64, :])
        nc.vector.tensor_add(dst[64:128, s0 : s0 + 512], m1[64:128, :], m2[64:128, :])

    def phase1(sc_i):
        s0 = sc_i * 512
        q_acc = [ps.tile([128, 512], F32, tag="obig", bufs=2, name=f"q_acc{h}") for h in range(2)]
        q_acc += [ps.tile([128, 512], F32, tag="s4", bufs=4, name=f"q_acc{h}") for h in (2, 3)]
        k_acc = ps.tile([128, 512], F32, tag="s4", bufs=4, name="k_acc")
        v_acc = ps.tile([128, 512], F32, tag="s4", bufs=4, name="v_acc")
        xt = None
        for k in range(NKBLK):
            if sc_i == 0:  # stream the remaining weight pieces behind the first
                if k == 2:
                    nc.sync.dma_start(wq_t[:, 2 * QC : 8 * QC], wq_sb_d[:, 2 * QC : 8 * QC])
                elif k == 4:
                    nc.sync.dma_start(wk_t[:, 16 * HD :], wk_sb_d[:, 16 * HD :])
                    nc.sync.dma_start(wv_t[:, 16 * HD :], wv_sb_d[:, 16 * HD :])
                elif k in (8, 16, 24):
                    nc.sync.dma_start(wq_t[:, k * QC : (k + 8) * QC],
                                      wq_sb_d[:, k * QC : (k + 8) * QC])
            if k % 4 == 0:
                # one batched DMA brings 4 k-blocks of x
                xt = sbw.tile([128, 4 * 512], BF16, tag="xt", bufs=3, name="xt")
                nc.sync.dma_start(
                    xt[:].rearrange("p (c s) -> p c s", c=4),
                    xT_r[:, k : k + 4, s0 : s0 + 512],
                )
            xk = xt[:, (k % 4) * 512 : (k % 4 + 1) * 512]
            st = k == 0
            sp = k == NKBLK - 1
            nc.tensor.matmul(k_acc[:], wk_t[:, k * HD : (k + 1) * HD], xk, start=st, stop=sp)
            nc.tensor.matmul(v_acc[:], wv_t[:, k * HD : (k + 1) * HD], xk, start=st, stop=sp)
            for h in range(HQ):
                wsl = wq_t[:, k * QC + h * 128 : k * QC + (h + 1) * 128]
                nc.tensor.matmul(q_acc[h][:], wsl, xk, start=st, stop=sp)
            if k == 2 and pend_transposes:
                pend_transposes.pop()()
        # drain all PSUM first (frees the banks for the next chunk's k-loop)
        cpy_k = rope_drain(k_acc)
        vtmp = sbw.tile([128, 512], BF16, tag="vtmp", bufs=2, name="vtmp")
        nc.vector.tensor_copy(vtmp[:], v_acc[:])
        cpy_q = [rope_drain(q_acc[h]) for h in range(HQ)]
        # k/v complete early (phase 2 needs kT/vnat first)
        rope_finish(kT, cpy_k, s0)

        def transposes(sc=sc_i, vt=vtmp):
            for i in range(4):
                j = 4 * sc + i
                tp = ps.tile([128, 512], BF16, tag="den", bufs=2, name="tp")
                nc.tensor.transpose(tp[:, 0:128], vt[:, i * 128 : (i + 1) * 128], ident_for(tc, sb))
                if i % 2 == 0:
                    nc.vector.tensor_copy(vnat[:, j * 128 : (j + 1) * 128], tp[:, 0:128])
                else:
                    nc.scalar.copy(vnat[:, j * 128 : (j + 1) * 128], tp[:, 0:128])

        pend_transposes.append(transposes)
        for h in range(HQ):
            rope_finish(qT[h], cpy_q[h], s0)

    # =================== phase 2: attention ===================
    # One head at a time over 512-query chunks. The j-loop is software-
    # pipelined at depth 2, and scores rotate through FOUR [128,512] PSUM
    # slots so the scores->exp->scores slot-reuse chain (retire + exp +
    # semaphores, ~1.7us round trip) has a 4-block reuse distance and the
    # PE stays the binding resource.
    def phase2(c, h):
        s0 = c * 512
        jmax = 4 * c + 3
        o_ps = ps.tile([128, 512], F32, tag="obig", bufs=2, name="o_ps")
        den_ps = ps.tile([128, 512], F32, tag="den", bufs=2, name="den_ps")

        def av(item):
            j, off, et = item
            nc.tensor.matmul(o_ps[:, off:512], vnat[:, j * 128 : (j + 1) * 128],
                             et[:, off:512], start=(j == 0), stop=(j == jmax))

        def den(item):
            j, off, et = item
            nc.tensor.matmul(den_ps[0:1, off:512], ones[:, 0:1],
                             et[:, off:512], start=(j == 0), stop=(j == jmax))

        pend_av = []
        pend_den = []
        for j in range(jmax + 1):
            off = 128 * max(0, j - 4 * c)  # columns below off are fully masked
            g = j - 4 * c
            s_ps = ps.tile([128, 512], F32, tag="s4", bufs=4, name="s_ps")
            nc.tensor.matmul(s_ps[:, off:512], kT[:, j * 128 : (j + 1) * 128],
                             qT[h][:, s0 + off : s0 + 512], start=True, stop=True)
            et = sbw.tile([128, 512], BF16, tag="et", bufs=10, name="et")
            nc.scalar.activation(
                et[:, off:512], s_ps[:, off:512],
                mybir.ActivationFunctionType.Exp, scale=SCALE,
            )
            if g >= 0:  # diagonal block: zero keys kk > s via tri mask
                # on Pool: the DVE queue carries the normalize chain, and an
                # in-order wait there would stall exp's et-slot guard
                nc.gpsimd.tensor_mul(et[:, off : off + 128], et[:, off : off + 128], tri[:])
            pend_av.append((j, off, et))
            pend_den.append((j, off, et))
            # AV trails scores by three blocks (clears the exp round-trip);
            # den, consumed only at the head tail, trails by six
            if len(pend_av) > 3:
                av(pend_av.pop(0))
            if len(pend_den) > 6:
                den(pend_den.pop(0))
        for item in pend_av:
            av(item)
        for item in pend_den:
            den(item)

        # normalize: qT[h][:, chunk] = o_ps / denom. The o copy frees the
        # PSUM bank early; 1/den broadcasts across partitions via a DRAM
        # roundtrip on the Pool DMA queue; DVE multiplies into bf16 qT.
        row = 1 + c * 4 + h
        o_sb = sbw.tile([128, 512], F32, tag="bcast", bufs=4, name="o_sb")
        nc.vector.tensor_copy(o_sb[:], o_ps[:])
        den_sb = sbw.tile([1, 512], F32, tag="den", bufs=4, name="den_sb")
        nc.vector.tensor_copy(den_sb[0:1, :], den_ps[0:1, :])
        rec_row = sbw.tile([1, 512], F32, tag="den", bufs=4, name="rec_row")
        rec_scr = sbw.tile([1, 512], F32, tag="den", bufs=4, name="rec_scr")
        nc.vector.reciprocal_approx_accurate(
            rec_row[0:1, :], den_sb[0:1, :], rec_scr[0:1, :]
        )
        nc.gpsimd.dma_start(dscr[row : row + 1, 0:512], rec_row[0:1, :])
        rec_bc = sbw.tile([128, 512], F32, tag="bcast", bufs=4, name="rec_bc")
        nc.gpsimd.dma_start(rec_bc[:], dscr[row : row + 1, 0:512].to_broadcast((128, 512)))
        nc.vector.tensor_mul(qT[h][:, s0 : s0 + 512], o_sb[:], rec_bc[:])

    oT = qT  # qT tiles hold the normalized attention output after phase2

    # =================== phase 3: output projection ===================
    wo_r = wo_sb_d.rearrange("p (c j) -> p c j", c=4)

    def phase3():
        for jb in range(HID // 128):
            wob = sbw.tile([128, 512], BF16, tag="wob", bufs=4, name="wob")
            nc.sync.dma_start(
                wob[:].rearrange("p (c j) -> p c j", c=4),
                wo_r[:, :, jb * 128 : (jb + 1) * 128],
            )
            y_ps = [ps.tile([128, 512], F32, tag="s4", bufs=4, name="y_ps") for _ in range(4)]
            for cb in range(4):
                for sc_i in range(4):
                    nc.tensor.matmul(
                        y_ps[sc_i][:],
                        wob[:, cb * 128 : (cb + 1) * 128],
                        oT[cb][:, sc_i * 512 : (sc_i + 1) * 512],
                        start=(cb == 0),
                        stop=(cb == 3),
                    )
            for sc_i in range(4):
                yst = sbw.tile([128, 512], BF16, tag="yst", bufs=8, name="yst")
                nc.vector.tensor_copy(yst[:], y_ps[sc_i][:])
                nc.sync.dma_start(
                    yT_d[jb * 128 : (jb + 1) * 128, sc_i * 512 : (sc_i + 1) * 512], yst[:]
                )

    pend_transposes = []
    for sc_i in range(4):
        phase1(sc_i)
    while pend_transposes:
        pend_transposes.pop()()
    for c in range(4):
        for h in range(HQ):
            phase2(c, h)
    phase3()


_IDENT = {}


def ident_for(tc, sb):
    if "t" not in _IDENT:
        from concourse.masks import make_identity

        ident = sb.tile([128, 128], BF16, name="ident")
        make_identity(tc.nc, ident)
        _IDENT["t"] = ident
    return _IDENT["t"]


_NC_CACHE = {}


def _get_nc():
    key = "v3"
    if key not in _NC_CACHE:
        _IDENT.clear()
        nc = bacc.Bacc("TRN2", target_bir_lowering=False, debug=False, num_devices=N_CORES)
        with TileContext(nc) as tc:
            with (
                tc.tile_pool(name="sb", bufs=1) as sb,
                tc.tile_pool(name="sbw", bufs=1) as sbw,
                tc.tile_pool(name="ps", bufs=1, space="PSUM") as ps,
            ):
                _build_body(tc, sb, sbw, ps)
        nc.compile()
        _NC_CACHE[key] = nc
    return _NC_CACHE[key]


_ROPE_PERM = np.concatenate([np.arange(0, 128, 2), np.arange(1, 128, 2)])


def _rope_tables(start_pos):
    freqs = 1.0 / (THETA ** (np.arange(0, HD, 2, dtype=np.float64) / HD))
    t = np.arange(start_pos, start_pos + SEQ, dtype=np.float64)
    ang = np.outer(t, freqs)  # [SEQ, 64]
    cosT = np.cos(ang).T.astype(np.float32)  # [64, SEQ]
    sinT = np.sin(ang).T.astype(np.float32)
    cc = np.ascontiguousarray(np.concatenate([cosT, cosT], axis=0))
    ss = np.ascontiguousarray(np.concatenate([sinT, sinT], axis=0))
    return cc, ss


def _to_kblock_layout(wT, cwidth):
    """[HID, cwidth] feature-major weight -> [128, NKBLK*cwidth] with k-block
    k at columns [k*cwidth, (k+1)*cwidth)."""
    return np.ascontiguousarray(
        wT.reshape(NKBLK, 128, cwidth).transpose(1, 0, 2).reshape(128, NKBLK * cwidth)
    )


def make_in_maps(x, wq, wk, wv, wo, start_pos):
    import ml_dtypes

    bf = ml_dtypes.bfloat16
    x = np.asarray(x, dtype=np.float32)
    wq = np.asarray(wq, dtype=np.float32)
    wk = np.asarray(wk, dtype=np.float32)
    wv = np.asarray(wv, dtype=np.float32)
    wo = np.asarray(wo, dtype=np.float32)
    sp = int(start_pos)

    xT = np.ascontiguousarray(x.T).astype(bf)
    cc, ss = _rope_tables(sp)
    woT = np.ascontiguousarray(wo.T)  # [in=c, out=j]
    ones_in = np.ones((128, 128), dtype=bf)
    tri_in = np.tril(np.ones((128, 128), dtype=np.float32)).T.astype(bf)  # tri[kk,i]=1 iff kk<=i

    in_maps = []
    for c in range(N_CORES):
        wq_c = wq[c * QC : (c + 1) * QC, :]  # [512, HID]
        wq_c = wq_c.reshape(HQ, HD, HID)[:, _ROPE_PERM, :].reshape(QC, HID)
        wk_c = wk[c * HD : (c + 1) * HD, :][_ROPE_PERM, :]  # [128, HID]
        wv_c = wv[c * HD : (c + 1) * HD, :]  # [128, HID]
        wq_sbm = _to_kblock_layout(np.ascontiguousarray(wq_c.T), QC)
        wk_sbm = _to_kblock_layout(np.ascontiguousarray(wk_c.T), HD)
        wv_sbm = _to_kblock_layout(np.ascontiguousarray(wv_c.T), HD)
        woT_c = woT[c * QC : (c + 1) * QC, :]  # [512, HID]
        wo_sbm = np.ascontiguousarray(
            woT_c.reshape(4, 128, HID).transpose(1, 0, 2).reshape(128, 4 * HID)
        )
        in_maps.append(
            {
                "xT": xT,
                "wq_sb": wq_sbm.astype(bf),
                "wk_sb": wk_sbm.astype(bf),
                "wv_sb": wv_sbm.astype(bf),
                "wo_sb": wo_sbm.astype(bf),
                "ones_in": ones_in,
                "tri_in": tri_in,
                "cc": cc,
                "ss": ss,
            }
        )
    return in_maps


def _assemble(results):
    acc = results[0]["yT"].astype(np.float32)
    for r in results[1:]:
        acc = acc + r["yT"].astype(np.float32)
    return np.ascontiguousarray(acc.T)


def _row0_expected(x, wv, wo):
    """Exact y[0]: query 0 attends only key 0, so o[0] is v[0] broadcast over
    the 4 q heads of each kv head; cheap host-side corruption check."""
    v0 = np.asarray(x[0], np.float64) @ np.asarray(wv, np.float64).T  # [1024]
    o0 = np.concatenate([v0[(h // HQ) * HD : (h // HQ + 1) * HD] for h in range(NQ)])
    return o0 @ np.asarray(wo, np.float64).T  # [4096]


def kernel(x, wq, wk, wv, wo, start_pos):
    nc = _get_nc()
    in_maps = make_in_maps(x, wq, wk, wv, wo, start_pos)
    y0 = _row0_expected(x, wv, wo)
    out = None
    for attempt in range(2):
        res = bass_utils.run_bass_kernel_spmd(nc, in_maps, core_ids=list(range(N_CORES)))
        out = _assemble(res.results)
        err0 = float(np.linalg.norm(out[0] - y0) / (np.linalg.norm(y0) + 1e-30))
        if np.isfinite(out).all() and err0 < 2e-2:
            break
        # a wedged device can corrupt a run silently; one retry clears it
    return out
